# revision 13
# baseline (speedup 1.0000x reference)
"""Trainium2 Bass kernel for nn_Attention_43413529428606 (linear attention
with l2-normed q/k, interleaved RoPE, mask, per-head power scaling).

v2: mask-compacted rows.  Masked rows contribute nothing (k,q masked; kv
only sums unmasked rows) so the host gathers each batch's unmasked rows
(~4.1k of 8192) and strides them across the 4 cores of that batch's
group; each core processes a fixed budget of 1152 rows (real rows
zero-padded, pad flag in maskC).  This cuts every row-proportional GEMM
by ~44% vs the 2048-row dense split.

Other changes vs v1: activation engine restricted to {Copy, Sqrt} (one
act-table set, no reload thrash; rsqrt = DVE reciprocal + Act sqrt),
element-wise work spread across DVE/Pool/Act, x DMA'd in c-chunks so the
first k-projection starts early, norms matmuls emitted one j-tile late,
output stored f16.

Self-contained: hardcodes all shapes; no sibling imports.
"""

import sys

for _p in ("/opt/trn_rl_repo",):
    if _p not in sys.path:
        sys.path.append(_p)

from contextlib import ExitStack

import numpy as np

import concourse.bass as bass
import concourse.bacc as bacc
import concourse.tile as tile
from concourse import mybir
from concourse.bass_utils import run_bass_kernel_spmd

F32 = mybir.dt.float32
F16 = mybir.dt.float16

DIM = 1024
H = 16
HD = 64
B = 2
C = 8192
ROPE_THETA = 10000.0

N_CORES = 8
R = 1152  # padded unmasked-row budget per core (~1037 real at seed 0)
NC_T = R // 128  # 9 c-tiles of 128 (phase A)
ST = 384  # phase-B supertile width
NST = R // ST  # 3
NCHUNK = 3  # x DMA chunks of ST columns
ND = DIM // 128  # 8 d-chunks
NJ = DIM // 128  # 8 j-tiles
NPAIR = H // 2  # 8 head pairs

Copy = mybir.ActivationFunctionType.Copy
Sqrt = mybir.ActivationFunctionType.Sqrt
MUL = mybir.AluOpType.mult
ADD = mybir.AluOpType.add


def build_nc(sim_mode=False, phases="ABC", reps=1, no_collective=False):
    nc = bacc.Bacc(
        "TRN2",
        target_bir_lowering=False,
        debug=False,
        num_devices=1 if sim_mode else N_CORES,
    )

    # ---- DRAM parameters (per-core shapes, fp16 data path) ----
    # x stored c-chunk-major: chunk i is xT[:, i*ST:(i+1)*ST], stacked on
    # axis 0 -> [NCHUNK*DIM, ST]; each chunk DMA is fully contiguous.
    xTc = nc.dram_tensor("xTc", [NCHUNK * DIM, ST], F16, kind="ExternalInput").ap()
    WkT = nc.dram_tensor("WkT", [DIM, DIM], F16, kind="ExternalInput").ap()
    WvT = nc.dram_tensor("WvT", [DIM, DIM], F16, kind="ExternalInput").ap()
    WqT = nc.dram_tensor("WqT", [DIM, DIM], F16, kind="ExternalInput").ap()
    WoT = nc.dram_tensor("WoT", [DIM, DIM], F16, kind="ExternalInput").ap()
    cosC = nc.dram_tensor("cosC", [R, HD], F16, kind="ExternalInput").ap()
    sinC = nc.dram_tensor("sinC", [R, HD], F16, kind="ExternalInput").ap()
    cosF = nc.dram_tensor("cosF", [128, R], F16, kind="ExternalInput").ap()
    sinF = nc.dram_tensor("sinF", [128, R], F16, kind="ExternalInput").ap()
    maskC = nc.dram_tensor("maskC", [128, NC_T], F32, kind="ExternalInput").ap()
    ind16T = nc.dram_tensor("ind16T", [DIM, 16], F16, kind="ExternalInput").ap()
    ind16 = nc.dram_tensor("ind16", [16, DIM], F16, kind="ExternalInput").ap()
    Pmat = nc.dram_tensor("Pmat", [128, 128], F16, kind="ExternalInput").ap()

    kv_in_d = nc.dram_tensor("kv_in_d", [128, NPAIR * 128], F16)
    kv_out_d = nc.dram_tensor("kv_out_d", [128, NPAIR * 128], F16)

    out_d = nc.dram_tensor("out", [DIM, R], F16, kind="ExternalOutput").ap()

    def blkview(dram_ap, csl):
        return dram_ap.rearrange("(t p) c -> p t c", p=128)[:, :, csl]

    with tile.TileContext(nc) as tc:
        with ExitStack() as ctx:
            consts = ctx.enter_context(tc.tile_pool(name="consts", bufs=1))
            kvblk_pool = ctx.enter_context(tc.tile_pool(name="kvblk", bufs=1))

            cosC_t = consts.tile([128, NC_T * HD], F16, tag="cosC")
            sinC_t = consts.tile([128, NC_T * HD], F16, tag="sinC")
            nc.scalar.dma_start(
                out=cosC_t[:].rearrange("p (t f) -> p t f", t=NC_T),
                in_=cosC[:].rearrange("(t p) f -> p t f", p=128),
            )
            nc.scalar.dma_start(
                out=sinC_t[:].rearrange("p (t f) -> p t f", t=NC_T),
                in_=sinC[:].rearrange("(t p) f -> p t f", p=128),
            )
            maskC_t = consts.tile([128, NC_T], F32, tag="maskC")
            ind16T_t = consts.tile([128, NJ * 16], F16, tag="ind16T")
            ind16_t = consts.tile([16, DIM], F16, tag="ind16")
            P_t = consts.tile([128, 128], F16, tag="Pmat")
            cosF_t = consts.tile([128, R], F16, tag="cosF")
            sinF_t = consts.tile([128, R], F16, tag="sinF")
            nc.scalar.dma_start(out=maskC_t[:], in_=maskC[:])
            nc.scalar.dma_start(
                out=ind16T_t[:].rearrange("p (t f) -> p t f", t=NJ),
                in_=ind16T[:].rearrange("(t p) f -> p t f", p=128),
            )
            nc.scalar.dma_start(out=ind16_t[:], in_=ind16[:])
            nc.scalar.dma_start(out=P_t[:], in_=Pmat[:])
            nc.scalar.dma_start(out=cosF_t[:], in_=cosF[:])
            nc.scalar.dma_start(out=sinF_t[:], in_=sinF[:])

            for _rep in range(reps):
              with ExitStack() as ctxX:
                xpool = ctxX.enter_context(tc.tile_pool(name="xpool", bufs=1))
                wpool = ctxX.enter_context(tc.tile_pool(name="wpool", bufs=1))

                xT_all = xpool.tile([128, ND * R], F16, tag="xT")
                wk_all = wpool.tile([128, ND * DIM], F16, tag="wk")
                wv_all = wpool.tile([128, ND * DIM], F16, tag="wv")
                wq_all = wpool.tile([128, ND * DIM], F16, tag="wq")
                wo_all = wpool.tile([128, ND * DIM], F16, tag="wo")

                xview = xT_all[:].rearrange("p (t c) -> p t c", t=ND)

                # x chunk 0 + wk first (gates the first k matmuls), then
                # the rest; sync and scalar queues in parallel.
                def load_xchunk(ci):
                    nc.sync.dma_start(
                        out=xview[:, :, ci * ST : (ci + 1) * ST],
                        in_=xTc[ci * DIM : (ci + 1) * DIM, :].rearrange(
                            "(t p) c -> p t c", p=128
                        ),
                    )

                def load_w(wt, wsrc, eng):
                    eng.dma_start(
                        out=wt[:].rearrange("p (t f) -> p t f", t=ND),
                        in_=wsrc[:].rearrange("(t p) f -> p t f", p=128),
                    )

                load_xchunk(0)
                load_w(wk_all, WkT, nc.scalar)
                load_w(wv_all, WvT, nc.scalar)
                load_xchunk(1)
                load_xchunk(2)
                load_w(wq_all, WqT, nc.scalar)
                load_w(wo_all, WoT, nc.scalar)

                def xsl(dc, csl):
                    lo = dc * R
                    return xT_all[:, lo + csl.start : lo + csl.stop]

                # ========= Phase A: k/v proj + process + kv Grams ==========
                with ExitStack() as ctxA:
                  if "A" in phases:
                    psA = ctxA.enter_context(
                        tc.tile_pool(name="psA", bufs=6, space="PSUM")
                    )
                    pskv = ctxA.enter_context(
                        tc.tile_pool(name="pskv", bufs=1, space="PSUM")
                    )
                    sbA = ctxA.enter_context(tc.tile_pool(name="sbA", bufs=2))
                    sb1 = ctxA.enter_context(tc.tile_pool(name="sb1", bufs=2))
                    smA = ctxA.enter_context(tc.tile_pool(name="smA", bufs=2))

                    kv_ps = pskv.tile([128, NPAIR * 128], F32, tag="kvps")
                    kv_pending = []

                    # On HW start=True zeroes the whole PSUM bank, so only
                    # the first pair written to each bank may carry it.
                    def _emit_kv(item):
                        ct_, khat_, v_ = item
                        for p in range(NPAIR):
                            ps_ = slice(p * 128, (p + 1) * 128)
                            nc.tensor.matmul(
                                kv_ps[:, ps_],
                                v_[:, ps_],
                                khat_[:, ps_],
                                start=(
                                    True
                                    if sim_mode
                                    else (ct_ == 0 and p % 4 == 0)
                                ),
                                stop=(
                                    True if sim_mode else (ct_ == NC_T - 1)
                                ),
                            )

                    for ct in range(NC_T):
                        cs = slice(ct * 128, (ct + 1) * 128)
                        # 512-wide psum halves, evicted as soon as each
                        # accumulation closes: the proj stream never waits
                        # on a psum bank (ring of 6 single-bank tiles).
                        k_sb = sbA.tile([128, DIM], F16, tag="k_sb")
                        v_sb = sbA.tile([128, DIM], F16, tag="v_sb")
                        for wt, sb_t, masked in (
                            (wk_all, k_sb, False),
                            (wv_all, v_sb, True),
                        ):
                            for half in range(2):
                                js = slice(half * 512, (half + 1) * 512)
                                h_ps = psA.tile([128, 512], F32, tag="proj_ps")
                                for dc in range(ND):
                                    nc.tensor.matmul(
                                        h_ps[:],
                                        xsl(dc, cs),
                                        wt[
                                            :, dc * DIM + js.start : dc * DIM + js.stop
                                        ],
                                        start=(dc == 0),
                                        stop=(dc == ND - 1),
                                    )
                                if masked:
                                    nc.scalar.activation(
                                        sb_t[:, js],
                                        h_ps[:],
                                        Copy,
                                        scale=maskC_t[:, ct : ct + 1],
                                    )
                                else:
                                    nc.scalar.activation(sb_t[:, js], h_ps[:], Copy)

                        sq = sbA.tile([128, DIM], F16, tag="sq")
                        nc.vector.tensor_mul(sq[:], k_sb[:], k_sb[:])
                        red = smA.tile([128, H], F32, tag="red")
                        nc.vector.tensor_reduce(
                            red[:],
                            sq[:].rearrange("p (h f) -> p h f", h=H),
                            mybir.AxisListType.X,
                            ADD,
                        )
                        # rsqrt = sqrt(1/(red+eps)); eps guards zero pads
                        rede = smA.tile([128, H], F32, tag="rede")
                        nc.vector.tensor_scalar_add(rede[:], red[:], 1e-4)
                        inv = smA.tile([128, H], F32, tag="inv")
                        nc.vector.reciprocal(inv[:], rede[:])
                        rs = smA.tile([128, H], F32, tag="rs")
                        nc.scalar.activation(rs[:], inv[:], Sqrt)
                        rsm = smA.tile([128, H], F32, tag="rsm")
                        nc.vector.tensor_scalar_mul(
                            rsm[:], rs[:], maskC_t[:, ct : ct + 1]
                        )

                        cosb = (
                            cosC_t[:, ct * HD : (ct + 1) * HD]
                            .unsqueeze(1)
                            .broadcast_to([128, H, HD])
                        )
                        sinb4 = (
                            sinC_t[:, ct * HD : (ct + 1) * HD]
                            .rearrange("p (g two) -> p g two", two=2)
                            .unsqueeze(1)
                            .broadcast_to([128, H, HD // 2, 2])
                        )
                        k3 = k_sb[:].rearrange("p (h f) -> p h f", h=H)
                        k_sw = k_sb[:].rearrange(
                            "p (h g two) -> p h g two", h=H, two=2
                        )[:, :, :, ::-1]

                        m1 = sb1.tile([128, DIM], F16, tag="m1")
                        nc.vector.tensor_tensor(
                            m1[:].rearrange("p (h f) -> p h f", h=H), k3, cosb, MUL
                        )
                        m2 = sb1.tile([128, DIM], F16, tag="m2")
                        nc.vector.tensor_tensor(
                            m2[:].rearrange("p (h g two) -> p h g two", h=H, two=2),
                            k_sw,
                            sinb4,
                            MUL,
                        )
                        s = sb1.tile([128, DIM], F16, tag="s")
                        nc.vector.tensor_tensor(s[:], m1[:], m2[:], ADD)
                        khat = sbA.tile([128, DIM], F16, tag="khat")
                        rsb = rsm[:].unsqueeze(2).broadcast_to([128, H, HD])
                        nc.gpsimd.tensor_tensor(
                            khat[:].rearrange("p (h f) -> p h f", h=H),
                            s[:].rearrange("p (h f) -> p h f", h=H),
                            rsb,
                            MUL,
                        )

                        # kv Grams are issued one iteration late (software
                        # pipelining) so PE never waits on the khat chain
                        kv_pending.append((ct, khat, v_sb))
                        if len(kv_pending) > 1:
                            _emit_kv(kv_pending.pop(0))

                    while kv_pending:
                        _emit_kv(kv_pending.pop(0))

                    # evict kv partials and run the collective
                    kv_sb = sbA.tile([128, NPAIR * 128], F16, tag="kv_sb")
                    nc.vector.tensor_copy(kv_sb[:], kv_ps[:])
                    nc.sync.dma_start(out=kv_in_d.ap(), in_=kv_sb[:])
                    if sim_mode or no_collective:
                        # stand-in for the AllReduce so TimelineSim can run
                        # (no_collective: HW timing diagnostic, wrong output)
                        nc.sync.dma_start(out=kv_out_d.ap(), in_=kv_in_d.ap())
                    else:
                        nc.gpsimd.collective_compute(
                            "AllReduce",
                            ADD,
                            replica_groups=[[0, 1, 2, 3], [4, 5, 6, 7]],
                            ins=[kv_in_d.ap().opt()],
                            outs=[kv_out_d.ap().opt()],
                        )

                # kvT: load reduced Grams (already f16), zero the cross-head
                # 64-blocks, then fold Wo once: kvWo[j, e] = sum_j' kvT[j', j]
                # * WoT[j', e].  Out-proj then contracts qh directly with
                # kvWo -- the per-supertile attention matmuls disappear.
                kvWo = kvblk_pool.tile([128, NJ * DIM], F16, tag="kvWo")
                if "C" in phases:
                    kvT = kvblk_pool.tile([128, NPAIR * 128], F16, tag="kvT")
                    kvf = kvblk_pool.tile([128, NPAIR * 128], F16, tag="kvf")
                    nc.scalar.dma_start(out=kvf[:], in_=kv_out_d.ap())
                    nc.gpsimd.memset(kvT[:], 0.0)
                    # top-left diag blocks of each pair, then bottom-right
                    nc.gpsimd.tensor_copy(
                        kvT[0:64, :].rearrange("p (t f) -> p t f", t=NPAIR)[
                            :, :, 0:64
                        ],
                        kvf[0:64, :].rearrange("p (t f) -> p t f", t=NPAIR)[
                            :, :, 0:64
                        ],
                    )
                    nc.gpsimd.tensor_copy(
                        kvT[64:128, :].rearrange("p (t f) -> p t f", t=NPAIR)[
                            :, :, 64:128
                        ],
                        kvf[64:128, :].rearrange("p (t f) -> p t f", t=NPAIR)[
                            :, :, 64:128
                        ],
                    )
                    with tc.tile_pool(name="psW", bufs=2, space="PSUM") as psW:
                        for jt in range(NJ):
                            w_ps = psW.tile([128, DIM], F32, tag="w_ps")
                            for half in range(2):
                                js = slice(half * 512, (half + 1) * 512)
                                nc.tensor.matmul(
                                    w_ps[:, js],
                                    kvT[:, jt * 128 : (jt + 1) * 128],
                                    wo_all[:, jt * DIM + js.start : jt * DIM + js.stop],
                                    start=True,
                                    stop=True,
                                )
                            if jt % 2 == 0:
                                nc.scalar.activation(
                                    kvWo[:, jt * DIM : (jt + 1) * DIM], w_ps[:], Copy
                                )
                            else:
                                nc.vector.tensor_copy(
                                    kvWo[:, jt * DIM : (jt + 1) * DIM], w_ps[:]
                                )

                # ==== Fused phase B+C: q proj/norm/rope + attn + out proj ===
                with ExitStack() as ctxB:
                  if "B" in phases and "C" in phases:
                    psB = ctxB.enter_context(
                        tc.tile_pool(name="psB", bufs=2, space="PSUM")
                    )
                    psRR = ctxB.enter_context(
                        tc.tile_pool(name="psRR", bufs=2, space="PSUM")
                    )
                    psN = ctxB.enter_context(
                        tc.tile_pool(name="psN", bufs=1, space="PSUM")
                    )
                    psAt = ctxB.enter_context(
                        tc.tile_pool(name="psAt", bufs=1, space="PSUM")
                    )
                    psO = ctxB.enter_context(
                        tc.tile_pool(name="psO", bufs=2, space="PSUM")
                    )
                    sbB = ctxB.enter_context(tc.tile_pool(name="sbB", bufs=3))
                    sbS = ctxB.enter_context(
                        tc.tile_pool(name="sbS", bufs=2 * NJ)
                    )
                    sbQ = ctxB.enter_context(tc.tile_pool(name="sbQ", bufs=2))
                    sbQH = ctxB.enter_context(
                        tc.tile_pool(name="sbQH", bufs=NST)
                    )
                    sbAt = ctxB.enter_context(
                        tc.tile_pool(name="sbAt", bufs=NJ + 2)
                    )

                    def _emit_attn_out(item):
                        ct_, qh_ = item
                        cs_ = slice(ct_ * ST, (ct_ + 1) * ST)
                        o_all = sbQ.tile([128, NJ * ST], F16, tag="o_all")
                        for et in range(NJ):
                            elo = et * 128
                            o_ps = psO.tile([128, ST], F32, tag="o_ps")
                            for jt in range(NJ):
                                nc.tensor.matmul(
                                    o_ps[:],
                                    kvWo[
                                        :, jt * DIM + elo : jt * DIM + elo + 128
                                    ],
                                    qh_[:, jt * ST : (jt + 1) * ST],
                                    start=(jt == 0),
                                    stop=(jt == NJ - 1),
                                )
                            nc.vector.tensor_copy(
                                o_all[:, et * ST : (et + 1) * ST], o_ps[:]
                            )
                        nc.scalar.dma_start(
                            out=blkview(out_d, cs_),
                            in_=o_all[:].rearrange("p (t c) -> p t c", t=NJ),
                        )

                    at_pending = []
                    for ct in range(NST):
                        cs = slice(ct * ST, (ct + 1) * ST)
                        norms_ps = psN.tile([16, ST], F32, tag="norms")
                        qh_all = sbQH.tile([128, NJ * ST], F16, tag="qhall")
                        q_sbs = []
                        sq_pending = []

                        def _emit_norms(item):
                            jt_, sq_ = item
                            nc.tensor.matmul(
                                norms_ps[:],
                                ind16T_t[:, jt_ * 16 : (jt_ + 1) * 16],
                                sq_[:],
                                start=(jt_ == 0),
                                stop=(jt_ == NJ - 1),
                            )

                        # pass 1: projections + squares + norm accumulation
                        # (norms matmuls one jt late: PE never waits on sq)
                        for jt in range(NJ):
                            jlo = jt * 128
                            q_ps = psB.tile([128, ST], F32, tag="q_ps")
                            for dc in range(ND):
                                nc.tensor.matmul(
                                    q_ps[:],
                                    wq_all[
                                        :, dc * DIM + jlo : dc * DIM + jlo + 128
                                    ],
                                    xsl(dc, cs),
                                    start=(dc == 0),
                                    stop=(dc == ND - 1),
                                )
                            q_sb = sbS.tile([128, ST], F16, tag="q_sb")
                            nc.scalar.activation(q_sb[:], q_ps[:], Copy)
                            sq = sbB.tile([128, ST], F16, tag="sqB")
                            nc.gpsimd.tensor_mul(sq[:], q_sb[:], q_sb[:])
                            sq_pending.append((jt, sq))
                            if len(sq_pending) > 1:
                                _emit_norms(sq_pending.pop(0))
                            q_sbs.append(q_sb)
                        while sq_pending:
                            _emit_norms(sq_pending.pop(0))

                        # rsqrt = sqrt(1/(norms+eps))
                        ne = sbB.tile([16, ST], F32, tag="ne")
                        nc.vector.tensor_scalar_add(ne[:], norms_ps[:], 1e-4)
                        inv16 = sbB.tile([16, ST], F32, tag="inv16")
                        nc.vector.reciprocal(inv16[:], ne[:])
                        rs16 = sbB.tile([16, ST], F16, tag="rs16")
                        nc.scalar.activation(rs16[:], inv16[:], Sqrt)

                        # pass 2: rot matmuls first (independent of rs16),
                        # then rep broadcast matmuls + the rope/scale chain
                        for jt in range(NJ):
                            rot_ps = psRR.tile([128, ST], F32, tag="rotrep")
                            nc.tensor.matmul(
                                rot_ps[:], P_t[:], q_sbs[jt][:],
                                start=True, stop=True,
                            )
                            rep_ps = psRR.tile([128, ST], F32, tag="rotrep")
                            nc.tensor.matmul(
                                rep_ps[:],
                                ind16_t[:, jt * 128 : (jt + 1) * 128],
                                rs16[:],
                                start=True,
                                stop=True,
                            )

                            t1 = sbB.tile([128, ST], F16, tag="t1")
                            nc.vector.tensor_tensor(
                                t1[:], q_sbs[jt][:], cosF_t[:, cs], MUL
                            )
                            t2 = sbB.tile([128, ST], F16, tag="t2")
                            nc.vector.tensor_tensor(
                                t2[:], rot_ps[:], sinF_t[:, cs], MUL
                            )
                            sB = sbB.tile([128, ST], F16, tag="sB")
                            nc.vector.tensor_tensor(sB[:], t1[:], t2[:], ADD)
                            nc.vector.tensor_tensor(
                                qh_all[:, jt * ST : (jt + 1) * ST],
                                sB[:],
                                rep_ps[:],
                                MUL,
                            )

                        at_pending.append((ct, qh_all))
                        if len(at_pending) > 2:
                            _emit_attn_out(at_pending.pop(0))

                    while at_pending:
                        _emit_attn_out(at_pending.pop(0))

    nc.compile()
    return nc


_NC_CACHE = None


def _get_nc():
    global _NC_CACHE
    if _NC_CACHE is None:
        _NC_CACHE = build_nc()
    return _NC_CACHE


def _row_assignment(mask):
    """Per-core unmasked row indices: batch group b gets cores 4b..4b+3,
    rows strided so counts differ by <=1."""
    rows_per_core = []
    for b in range(B):
        idx = np.where(np.asarray(mask[b]) != 0)[0]
        for cc in range(N_CORES // B):
            rows_per_core.append(idx[cc :: N_CORES // B])
    return rows_per_core


def make_in_maps(x, mask, Wq, Wk, Wv, Wo, norm_const):
    x = np.asarray(x, np.float32)
    mask = np.asarray(mask)
    Wq = np.asarray(Wq, np.float32)
    Wk = np.asarray(Wk, np.float32)
    Wv = np.asarray(Wv, np.float32)
    Wo = np.asarray(Wo, np.float32)
    norm_const = np.asarray(norm_const, np.float32).reshape(H)

    sig = 1.0 / (1.0 + np.exp(-norm_const.astype(np.float64)))
    svec = np.float64(C) ** (-sig)  # [H]
    s_cols = np.repeat(svec, HD)  # [DIM]

    f16 = np.float16
    WkT = np.ascontiguousarray(Wk.T).astype(f16)
    WvT = np.ascontiguousarray((Wv * s_cols[:, None].astype(np.float32)).T).astype(
        f16
    )
    WqT = np.ascontiguousarray(Wq.T).astype(f16)
    WoT = np.ascontiguousarray(Wo.T).astype(f16)

    inv_freq = 1.0 / (
        ROPE_THETA ** (np.arange(0, HD, 2, dtype=np.float64) / HD)
    )  # [32]
    freq_of_j = np.repeat(inv_freq, 2)  # [64] interleaved

    ind16T = np.zeros((DIM, 16), f16)
    for jt in range(NJ):
        for kk in range(128):
            ind16T[jt * 128 + kk, 2 * jt + (kk >= 64)] = 1.0

    ind16 = np.zeros((16, DIM), f16)
    for jt in range(NJ):
        for m in range(128):
            ind16[2 * jt + (m >= 64), jt * 128 + m] = 1.0

    Pmat = np.zeros((128, 128), f16)
    for i in range(64):
        Pmat[2 * i + 1, 2 * i] = -1.0  # out[2i] = -q[2i+1]
        Pmat[2 * i, 2 * i + 1] = 1.0  # out[2i+1] = q[2i]

    rows_per_core = _row_assignment(mask)

    in_maps = []
    for core in range(N_CORES):
        b = core // (N_CORES // B)
        rows = rows_per_core[core]
        n = len(rows)
        assert n <= R, f"core {core}: {n} unmasked rows exceed budget {R}"

        pos = np.zeros(R, np.float64)
        pos[:n] = rows

        xc = np.zeros((R, DIM), np.float32)
        xc[:n] = x[b, rows, :]
        xT = xc.T.astype(f16)  # [DIM, R]
        xTc = np.concatenate(
            [xT[:, i * ST : (i + 1) * ST] for i in range(NCHUNK)], axis=0
        )
        xTc = np.ascontiguousarray(xTc)

        angC = pos[:, None] * freq_of_j[None, :]  # [R, 64]
        cosCc = np.cos(angC).astype(f16)
        sinCc = np.sin(angC).astype(np.float32)
        # sign fold for the swap formulation: even j -> -sin, odd j -> +sin
        sinCc[:, 0::2] *= -1.0
        sinCc = sinCc.astype(f16)

        angF = freq_of_j[:, None] * pos[None, :]  # [64, R]
        angF2 = np.concatenate([angF, angF], axis=0)  # [128, R]
        cosFc = np.cos(angF2).astype(f16)
        sinFc = np.sin(angF2).astype(f16)

        flags = np.zeros(R, np.float32)
        flags[:n] = 1.0
        maskCc = np.ascontiguousarray(flags.reshape(NC_T, 128).T)  # [128, NC_T]

        in_maps.append(
            {
                "xTc": xTc,
                "WkT": WkT,
                "WvT": WvT,
                "WqT": WqT,
                "WoT": WoT,
                "cosC": cosCc,
                "sinC": sinCc,
                "cosF": cosFc,
                "sinF": sinFc,
                "maskC": maskCc,
                "ind16T": ind16T,
                "ind16": ind16,
                "Pmat": Pmat,
            }
        )
    return in_maps


def assemble_output(results, mask):
    rows_per_core = _row_assignment(mask)
    out = np.zeros((B, C, DIM), np.float32)
    for core in range(N_CORES):
        b = core // (N_CORES // B)
        rows = rows_per_core[core]
        n = len(rows)
        o = results[core]["out"]  # [DIM, R] f16
        out[b, rows, :] = o[:, :n].T.astype(np.float32)
    return out


def kernel(x, mask, Wq, Wk, Wv, Wo, norm_const):
    nc = _get_nc()
    in_maps = make_in_maps(x, mask, Wq, Wk, Wv, Wo, norm_const)
    res = run_bass_kernel_spmd(nc, in_maps, list(range(N_CORES)))
    return assemble_output(res.results, mask)


# revision 16
# speedup vs baseline: 1.0496x; 1.0496x over previous
"""Trainium2 Bass kernel for nn_Attention_43413529428606 (linear attention
with l2-normed q/k, interleaved RoPE, mask, per-head power scaling).

v2: mask-compacted rows.  Masked rows contribute nothing (k,q masked; kv
only sums unmasked rows) so the host gathers each batch's unmasked rows
(~4.1k of 8192) and strides them across the 4 cores of that batch's
group; each core processes a fixed budget of 1152 rows (real rows
zero-padded, pad flag in maskC).  This cuts every row-proportional GEMM
by ~44% vs the 2048-row dense split.

Other changes vs v1: activation engine restricted to {Copy, Sqrt} (one
act-table set, no reload thrash; rsqrt = DVE reciprocal + Act sqrt),
element-wise work spread across DVE/Pool/Act, x DMA'd in c-chunks so the
first k-projection starts early, norms matmuls emitted one j-tile late,
output stored f16.

Self-contained: hardcodes all shapes; no sibling imports.
"""

import sys

for _p in ("/opt/trn_rl_repo",):
    if _p not in sys.path:
        sys.path.append(_p)

from contextlib import ExitStack

import numpy as np

import concourse.bass as bass
import concourse.bacc as bacc
import concourse.tile as tile
from concourse import mybir
from concourse.bass_utils import run_bass_kernel_spmd

F32 = mybir.dt.float32
F16 = mybir.dt.float16

DIM = 1024
H = 16
HD = 64
B = 2
C = 8192
ROPE_THETA = 10000.0

N_CORES = 8
R = 1152  # padded unmasked-row budget per core (~1037 real at seed 0)
NC_T = R // 128  # 9 c-tiles of 128 (phase A)
ST = 384  # phase-B supertile width
NST = R // ST  # 3
NCHUNK = 3  # x DMA chunks of ST columns
ND = DIM // 128  # 8 d-chunks
NJ = DIM // 128  # 8 j-tiles
NPAIR = H // 2  # 8 head pairs

Copy = mybir.ActivationFunctionType.Copy
Sqrt = mybir.ActivationFunctionType.Sqrt
MUL = mybir.AluOpType.mult
ADD = mybir.AluOpType.add


def build_nc(sim_mode=False, phases="ABC", reps=1, no_collective=False):
    nc = bacc.Bacc(
        "TRN2",
        target_bir_lowering=False,
        debug=False,
        num_devices=1 if sim_mode else N_CORES,
    )

    # ---- DRAM parameters (per-core shapes, fp16 data path) ----
    # x stored c-chunk-major: chunk i is xT[:, i*ST:(i+1)*ST], stacked on
    # axis 0 -> [NCHUNK*DIM, ST]; each chunk DMA is fully contiguous.
    xTc = nc.dram_tensor("xTc", [NCHUNK * DIM, ST], F16, kind="ExternalInput").ap()
    WkT = nc.dram_tensor("WkT", [DIM, DIM], F16, kind="ExternalInput").ap()
    WvT = nc.dram_tensor("WvT", [DIM, DIM], F16, kind="ExternalInput").ap()
    WqT = nc.dram_tensor("WqT", [DIM, DIM], F16, kind="ExternalInput").ap()
    WoT = nc.dram_tensor("WoT", [DIM, DIM], F16, kind="ExternalInput").ap()
    cosC = nc.dram_tensor("cosC", [R, HD], F16, kind="ExternalInput").ap()
    sinC = nc.dram_tensor("sinC", [R, HD], F16, kind="ExternalInput").ap()
    cosF = nc.dram_tensor("cosF", [128, R], F16, kind="ExternalInput").ap()
    sinF = nc.dram_tensor("sinF", [128, R], F16, kind="ExternalInput").ap()
    maskC = nc.dram_tensor("maskC", [128, NC_T], F32, kind="ExternalInput").ap()
    ind16T = nc.dram_tensor("ind16T", [DIM, 16], F16, kind="ExternalInput").ap()
    ind16 = nc.dram_tensor("ind16", [16, DIM], F16, kind="ExternalInput").ap()
    Pmat = nc.dram_tensor("Pmat", [128, 128], F16, kind="ExternalInput").ap()

    kv_in_d = nc.dram_tensor("kv_in_d", [128, NPAIR * 128], F16)
    kv_out_d = nc.dram_tensor("kv_out_d", [128, NPAIR * 128], F16)

    out_d = nc.dram_tensor("out", [DIM, R], F16, kind="ExternalOutput").ap()

    def blkview(dram_ap, csl):
        return dram_ap.rearrange("(t p) c -> p t c", p=128)[:, :, csl]

    with tile.TileContext(nc) as tc:
        with ExitStack() as ctx:
            consts = ctx.enter_context(tc.tile_pool(name="consts", bufs=1))
            kvblk_pool = ctx.enter_context(tc.tile_pool(name="kvblk", bufs=1))

            cosC_t = consts.tile([128, NC_T * HD], F16, tag="cosC")
            sinC_t = consts.tile([128, NC_T * HD], F16, tag="sinC")
            nc.scalar.dma_start(
                out=cosC_t[:].rearrange("p (t f) -> p t f", t=NC_T),
                in_=cosC[:].rearrange("(t p) f -> p t f", p=128),
            )
            nc.scalar.dma_start(
                out=sinC_t[:].rearrange("p (t f) -> p t f", t=NC_T),
                in_=sinC[:].rearrange("(t p) f -> p t f", p=128),
            )
            maskC_t = consts.tile([128, NC_T], F32, tag="maskC")
            ind16T_t = consts.tile([128, NJ * 16], F16, tag="ind16T")
            ind16_t = consts.tile([16, DIM], F16, tag="ind16")
            P_t = consts.tile([128, 128], F16, tag="Pmat")
            cosF_t = consts.tile([128, R], F16, tag="cosF")
            sinF_t = consts.tile([128, R], F16, tag="sinF")
            nc.scalar.dma_start(out=maskC_t[:], in_=maskC[:])
            nc.scalar.dma_start(
                out=ind16T_t[:].rearrange("p (t f) -> p t f", t=NJ),
                in_=ind16T[:].rearrange("(t p) f -> p t f", p=128),
            )
            nc.scalar.dma_start(out=ind16_t[:], in_=ind16[:])
            nc.scalar.dma_start(out=P_t[:], in_=Pmat[:])
            nc.scalar.dma_start(out=cosF_t[:], in_=cosF[:])
            nc.scalar.dma_start(out=sinF_t[:], in_=sinF[:])

            for _rep in range(reps):
              with ExitStack() as ctxX:
                xpool = ctxX.enter_context(tc.tile_pool(name="xpool", bufs=1))
                wpool = ctxX.enter_context(tc.tile_pool(name="wpool", bufs=1))

                xT_all = xpool.tile([128, ND * R], F16, tag="xT")
                wk_all = wpool.tile([128, ND * DIM], F16, tag="wk")
                wv_all = wpool.tile([128, ND * DIM], F16, tag="wv")
                wq_all = wpool.tile([128, ND * DIM], F16, tag="wq")
                wo_all = wpool.tile([128, ND * DIM], F16, tag="wo")

                xview = xT_all[:].rearrange("p (t c) -> p t c", t=ND)

                # x chunk 0 + wk first (gates the first k matmuls), then
                # the rest; sync and scalar queues in parallel.
                def load_xchunk(ci):
                    nc.sync.dma_start(
                        out=xview[:, :, ci * ST : (ci + 1) * ST],
                        in_=xTc[ci * DIM : (ci + 1) * DIM, :].rearrange(
                            "(t p) c -> p t c", p=128
                        ),
                    )

                def load_w(wt, wsrc, eng):
                    eng.dma_start(
                        out=wt[:].rearrange("p (t f) -> p t f", t=ND),
                        in_=wsrc[:].rearrange("(t p) f -> p t f", p=128),
                    )

                load_xchunk(0)
                load_w(wk_all, WkT, nc.scalar)
                load_w(wv_all, WvT, nc.scalar)
                load_xchunk(1)
                load_xchunk(2)
                load_w(wq_all, WqT, nc.scalar)
                load_w(wo_all, WoT, nc.scalar)

                def xsl(dc, csl):
                    lo = dc * R
                    return xT_all[:, lo + csl.start : lo + csl.stop]

                # ========= Phase A: k/v proj + process + kv Grams ==========
                with ExitStack() as ctxA:
                  if "A" in phases:
                    psA = ctxA.enter_context(
                        tc.tile_pool(name="psA", bufs=3, space="PSUM")
                    )
                    pskv = ctxA.enter_context(
                        tc.tile_pool(name="pskv", bufs=1, space="PSUM")
                    )
                    sbA = ctxA.enter_context(tc.tile_pool(name="sbA", bufs=3))
                    sb1 = ctxA.enter_context(tc.tile_pool(name="sb1", bufs=2))
                    smA = ctxA.enter_context(tc.tile_pool(name="smA", bufs=2))

                    kv_ps = pskv.tile([128, NPAIR * 128], F32, tag="kvps")
                    kv_pending = []

                    # On HW start=True zeroes the whole PSUM bank, so only
                    # the first pair written to each bank may carry it.
                    def _emit_kv(item):
                        ct_, khat_, v_ = item
                        for p in range(NPAIR):
                            ps_ = slice(p * 128, (p + 1) * 128)
                            nc.tensor.matmul(
                                kv_ps[:, ps_],
                                v_[:, ps_],
                                khat_[:, ps_],
                                start=(
                                    True
                                    if sim_mode
                                    else (ct_ == 0 and p % 4 == 0)
                                ),
                                stop=(
                                    True if sim_mode else (ct_ == NC_T - 1)
                                ),
                            )

                    for ct in range(NC_T):
                        cs = slice(ct * 128, (ct + 1) * 128)
                        k_ps = psA.tile([128, DIM], F32, tag="proj_ps")
                        v_ps = psA.tile([128, DIM], F32, tag="proj_ps")
                        for half in range(2):
                            js = slice(half * 512, (half + 1) * 512)
                            for dc in range(ND):
                                nc.tensor.matmul(
                                    k_ps[:, js],
                                    xsl(dc, cs),
                                    wk_all[
                                        :, dc * DIM + js.start : dc * DIM + js.stop
                                    ],
                                    start=(dc == 0),
                                    stop=(dc == ND - 1),
                                )
                        for half in range(2):
                            js = slice(half * 512, (half + 1) * 512)
                            for dc in range(ND):
                                nc.tensor.matmul(
                                    v_ps[:, js],
                                    xsl(dc, cs),
                                    wv_all[
                                        :, dc * DIM + js.start : dc * DIM + js.stop
                                    ],
                                    start=(dc == 0),
                                    stop=(dc == ND - 1),
                                )

                        # k evict first: it gates the norm/rope chain
                        k_sb = sbA.tile([128, DIM], F16, tag="k_sb")
                        nc.scalar.activation(k_sb[:], k_ps[:], Copy)
                        # v evict with mask fold (per-partition scale)
                        v_sb = sbA.tile([128, DIM], F16, tag="v_sb")
                        nc.scalar.activation(
                            v_sb[:], v_ps[:], Copy, scale=maskC_t[:, ct : ct + 1]
                        )

                        sq = sbA.tile([128, DIM], F16, tag="sq")
                        nc.vector.tensor_mul(sq[:], k_sb[:], k_sb[:])
                        red = smA.tile([128, H], F32, tag="red")
                        nc.vector.tensor_reduce(
                            red[:],
                            sq[:].rearrange("p (h f) -> p h f", h=H),
                            mybir.AxisListType.X,
                            ADD,
                        )
                        # rsqrt = sqrt(1/(red+eps)); eps guards zero pads
                        rede = smA.tile([128, H], F32, tag="rede")
                        nc.vector.tensor_scalar_add(rede[:], red[:], 1e-4)
                        inv = smA.tile([128, H], F32, tag="inv")
                        nc.vector.reciprocal(inv[:], rede[:])
                        rs = smA.tile([128, H], F32, tag="rs")
                        nc.scalar.activation(rs[:], inv[:], Sqrt)
                        rsm = smA.tile([128, H], F32, tag="rsm")
                        nc.vector.tensor_scalar_mul(
                            rsm[:], rs[:], maskC_t[:, ct : ct + 1]
                        )

                        cosb = (
                            cosC_t[:, ct * HD : (ct + 1) * HD]
                            .unsqueeze(1)
                            .broadcast_to([128, H, HD])
                        )
                        sinb4 = (
                            sinC_t[:, ct * HD : (ct + 1) * HD]
                            .rearrange("p (g two) -> p g two", two=2)
                            .unsqueeze(1)
                            .broadcast_to([128, H, HD // 2, 2])
                        )
                        k3 = k_sb[:].rearrange("p (h f) -> p h f", h=H)
                        k_sw = k_sb[:].rearrange(
                            "p (h g two) -> p h g two", h=H, two=2
                        )[:, :, :, ::-1]

                        m1 = sb1.tile([128, DIM], F16, tag="m1")
                        nc.vector.tensor_tensor(
                            m1[:].rearrange("p (h f) -> p h f", h=H), k3, cosb, MUL
                        )
                        m2 = sb1.tile([128, DIM], F16, tag="m2")
                        nc.gpsimd.tensor_tensor(
                            m2[:].rearrange("p (h g two) -> p h g two", h=H, two=2),
                            k_sw,
                            sinb4,
                            MUL,
                        )
                        s = sb1.tile([128, DIM], F16, tag="s")
                        nc.vector.tensor_tensor(s[:], m1[:], m2[:], ADD)
                        khat = sbA.tile([128, DIM], F16, tag="khat")
                        rsb = rsm[:].unsqueeze(2).broadcast_to([128, H, HD])
                        nc.vector.tensor_tensor(
                            khat[:].rearrange("p (h f) -> p h f", h=H),
                            s[:].rearrange("p (h f) -> p h f", h=H),
                            rsb,
                            MUL,
                        )

                        # kv Grams are issued one iteration late (software
                        # pipelining) so PE never waits on the khat chain
                        kv_pending.append((ct, khat, v_sb))
                        if len(kv_pending) > 2:
                            _emit_kv(kv_pending.pop(0))

                    while kv_pending:
                        _emit_kv(kv_pending.pop(0))

                    # evict kv partials and run the collective
                    kv_sb = sbA.tile([128, NPAIR * 128], F16, tag="kv_sb")
                    nc.vector.tensor_copy(kv_sb[:], kv_ps[:])
                    nc.sync.dma_start(out=kv_in_d.ap(), in_=kv_sb[:])
                    if sim_mode or no_collective:
                        # stand-in for the AllReduce so TimelineSim can run
                        # (no_collective: HW timing diagnostic, wrong output)
                        nc.sync.dma_start(out=kv_out_d.ap(), in_=kv_in_d.ap())
                    else:
                        nc.gpsimd.collective_compute(
                            "AllReduce",
                            ADD,
                            replica_groups=[[0, 1, 2, 3], [4, 5, 6, 7]],
                            ins=[kv_in_d.ap().opt()],
                            outs=[kv_out_d.ap().opt()],
                        )

                # kvT: load reduced Grams (already f16), zero the cross-head
                # 64-blocks, then fold Wo once: kvWo[j, e] = sum_j' kvT[j', j]
                # * WoT[j', e].  Out-proj then contracts qh directly with
                # kvWo -- the per-supertile attention matmuls disappear.
                kvWo = kvblk_pool.tile([128, NJ * DIM], F16, tag="kvWo")
                if "C" in phases:
                    kvT = kvblk_pool.tile([128, NPAIR * 128], F16, tag="kvT")
                    kvf = kvblk_pool.tile([128, NPAIR * 128], F16, tag="kvf")
                    nc.scalar.dma_start(out=kvf[:], in_=kv_out_d.ap())
                    nc.gpsimd.memset(kvT[:], 0.0)
                    # top-left diag blocks of each pair, then bottom-right
                    nc.gpsimd.tensor_copy(
                        kvT[0:64, :].rearrange("p (t f) -> p t f", t=NPAIR)[
                            :, :, 0:64
                        ],
                        kvf[0:64, :].rearrange("p (t f) -> p t f", t=NPAIR)[
                            :, :, 0:64
                        ],
                    )
                    nc.gpsimd.tensor_copy(
                        kvT[64:128, :].rearrange("p (t f) -> p t f", t=NPAIR)[
                            :, :, 64:128
                        ],
                        kvf[64:128, :].rearrange("p (t f) -> p t f", t=NPAIR)[
                            :, :, 64:128
                        ],
                    )
                    with tc.tile_pool(name="psW", bufs=2, space="PSUM") as psW:
                        for jt in range(NJ):
                            w_ps = psW.tile([128, DIM], F32, tag="w_ps")
                            for half in range(2):
                                js = slice(half * 512, (half + 1) * 512)
                                nc.tensor.matmul(
                                    w_ps[:, js],
                                    kvT[:, jt * 128 : (jt + 1) * 128],
                                    wo_all[:, jt * DIM + js.start : jt * DIM + js.stop],
                                    start=True,
                                    stop=True,
                                )
                            if jt % 2 == 0:
                                nc.scalar.activation(
                                    kvWo[:, jt * DIM : (jt + 1) * DIM], w_ps[:], Copy
                                )
                            else:
                                nc.vector.tensor_copy(
                                    kvWo[:, jt * DIM : (jt + 1) * DIM], w_ps[:]
                                )

                # ==== Fused phase B+C: q proj/norm/rope + attn + out proj ===
                with ExitStack() as ctxB:
                  if "B" in phases and "C" in phases:
                    psB = ctxB.enter_context(
                        tc.tile_pool(name="psB", bufs=2, space="PSUM")
                    )
                    psRR = ctxB.enter_context(
                        tc.tile_pool(name="psRR", bufs=2, space="PSUM")
                    )
                    psN = ctxB.enter_context(
                        tc.tile_pool(name="psN", bufs=1, space="PSUM")
                    )
                    psAt = ctxB.enter_context(
                        tc.tile_pool(name="psAt", bufs=1, space="PSUM")
                    )
                    psO = ctxB.enter_context(
                        tc.tile_pool(name="psO", bufs=2, space="PSUM")
                    )
                    sbB = ctxB.enter_context(tc.tile_pool(name="sbB", bufs=3))
                    sbS = ctxB.enter_context(
                        tc.tile_pool(name="sbS", bufs=2 * NJ)
                    )
                    sbQ = ctxB.enter_context(tc.tile_pool(name="sbQ", bufs=2))
                    sbQH = ctxB.enter_context(
                        tc.tile_pool(name="sbQH", bufs=NST)
                    )
                    sbAt = ctxB.enter_context(
                        tc.tile_pool(name="sbAt", bufs=NJ + 2)
                    )

                    def _emit_attn_out(item):
                        ct_, qh_ = item
                        cs_ = slice(ct_ * ST, (ct_ + 1) * ST)
                        o_all = sbQ.tile([128, NJ * ST], F16, tag="o_all")
                        for et in range(NJ):
                            elo = et * 128
                            o_ps = psO.tile([128, ST], F32, tag="o_ps")
                            for jt in range(NJ):
                                nc.tensor.matmul(
                                    o_ps[:],
                                    kvWo[
                                        :, jt * DIM + elo : jt * DIM + elo + 128
                                    ],
                                    qh_[:, jt * ST : (jt + 1) * ST],
                                    start=(jt == 0),
                                    stop=(jt == NJ - 1),
                                )
                            if et % 2 == 0:
                                nc.scalar.activation(
                                    o_all[:, et * ST : (et + 1) * ST], o_ps[:], Copy
                                )
                            else:
                                nc.vector.tensor_copy(
                                    o_all[:, et * ST : (et + 1) * ST], o_ps[:]
                                )
                        nc.scalar.dma_start(
                            out=blkview(out_d, cs_),
                            in_=o_all[:].rearrange("p (t c) -> p t c", t=NJ),
                        )

                    at_pending = []
                    for ct in range(NST):
                        cs = slice(ct * ST, (ct + 1) * ST)
                        norms_ps = psN.tile([16, ST], F32, tag="norms")
                        qh_all = sbQH.tile([128, NJ * ST], F16, tag="qhall")
                        q_sbs = []
                        sq_pending = []

                        def _emit_norms(item):
                            jt_, sq_ = item
                            nc.tensor.matmul(
                                norms_ps[:],
                                ind16T_t[:, jt_ * 16 : (jt_ + 1) * 16],
                                sq_[:],
                                start=(jt_ == 0),
                                stop=(jt_ == NJ - 1),
                            )

                        # pass 1: projections + squares + norm accumulation
                        # (norms matmuls one jt late: PE never waits on sq)
                        for jt in range(NJ):
                            jlo = jt * 128
                            q_ps = psB.tile([128, ST], F32, tag="q_ps")
                            for dc in range(ND):
                                nc.tensor.matmul(
                                    q_ps[:],
                                    wq_all[
                                        :, dc * DIM + jlo : dc * DIM + jlo + 128
                                    ],
                                    xsl(dc, cs),
                                    start=(dc == 0),
                                    stop=(dc == ND - 1),
                                )
                            q_sb = sbS.tile([128, ST], F16, tag="q_sb")
                            nc.scalar.activation(q_sb[:], q_ps[:], Copy)
                            sq = sbB.tile([128, ST], F16, tag="sqB")
                            nc.vector.tensor_mul(sq[:], q_sb[:], q_sb[:])
                            sq_pending.append((jt, sq))
                            if len(sq_pending) > 1:
                                _emit_norms(sq_pending.pop(0))
                            q_sbs.append(q_sb)
                        while sq_pending:
                            _emit_norms(sq_pending.pop(0))

                        # rsqrt = sqrt(1/(norms+eps))
                        ne = sbB.tile([16, ST], F32, tag="ne")
                        nc.vector.tensor_scalar_add(ne[:], norms_ps[:], 1e-4)
                        inv16 = sbB.tile([16, ST], F32, tag="inv16")
                        nc.vector.reciprocal(inv16[:], ne[:])
                        rs16 = sbB.tile([16, ST], F16, tag="rs16")
                        nc.scalar.activation(rs16[:], inv16[:], Sqrt)

                        # pass 2: rot matmuls first (independent of rs16),
                        # then rep broadcast matmuls + the rope/scale chain
                        for jt in range(NJ):
                            rot_ps = psRR.tile([128, ST], F32, tag="rotrep")
                            nc.tensor.matmul(
                                rot_ps[:], P_t[:], q_sbs[jt][:],
                                start=True, stop=True,
                            )
                            rep_ps = psRR.tile([128, ST], F32, tag="rotrep")
                            nc.tensor.matmul(
                                rep_ps[:],
                                ind16_t[:, jt * 128 : (jt + 1) * 128],
                                rs16[:],
                                start=True,
                                stop=True,
                            )

                            t1 = sbB.tile([128, ST], F16, tag="t1")
                            nc.vector.tensor_tensor(
                                t1[:], q_sbs[jt][:], cosF_t[:, cs], MUL
                            )
                            t2 = sbB.tile([128, ST], F16, tag="t2")
                            nc.vector.tensor_tensor(
                                t2[:], rot_ps[:], sinF_t[:, cs], MUL
                            )
                            sB = sbB.tile([128, ST], F16, tag="sB")
                            nc.vector.tensor_tensor(sB[:], t1[:], t2[:], ADD)
                            nc.vector.tensor_tensor(
                                qh_all[:, jt * ST : (jt + 1) * ST],
                                sB[:],
                                rep_ps[:],
                                MUL,
                            )

                        at_pending.append((ct, qh_all))
                        if len(at_pending) > 1:
                            _emit_attn_out(at_pending.pop(0))

                    while at_pending:
                        _emit_attn_out(at_pending.pop(0))

    nc.compile()
    return nc


_NC_CACHE = None


def _get_nc():
    global _NC_CACHE
    if _NC_CACHE is None:
        _NC_CACHE = build_nc()
    return _NC_CACHE


def _row_assignment(mask):
    """Per-core unmasked row indices: batch group b gets cores 4b..4b+3,
    rows strided so counts differ by <=1."""
    rows_per_core = []
    for b in range(B):
        idx = np.where(np.asarray(mask[b]) != 0)[0]
        for cc in range(N_CORES // B):
            rows_per_core.append(idx[cc :: N_CORES // B])
    return rows_per_core


def make_in_maps(x, mask, Wq, Wk, Wv, Wo, norm_const):
    x = np.asarray(x, np.float32)
    mask = np.asarray(mask)
    Wq = np.asarray(Wq, np.float32)
    Wk = np.asarray(Wk, np.float32)
    Wv = np.asarray(Wv, np.float32)
    Wo = np.asarray(Wo, np.float32)
    norm_const = np.asarray(norm_const, np.float32).reshape(H)

    sig = 1.0 / (1.0 + np.exp(-norm_const.astype(np.float64)))
    svec = np.float64(C) ** (-sig)  # [H]
    s_cols = np.repeat(svec, HD)  # [DIM]

    f16 = np.float16
    WkT = np.ascontiguousarray(Wk.T).astype(f16)
    WvT = np.ascontiguousarray((Wv * s_cols[:, None].astype(np.float32)).T).astype(
        f16
    )
    WqT = np.ascontiguousarray(Wq.T).astype(f16)
    WoT = np.ascontiguousarray(Wo.T).astype(f16)

    inv_freq = 1.0 / (
        ROPE_THETA ** (np.arange(0, HD, 2, dtype=np.float64) / HD)
    )  # [32]
    freq_of_j = np.repeat(inv_freq, 2)  # [64] interleaved

    ind16T = np.zeros((DIM, 16), f16)
    for jt in range(NJ):
        for kk in range(128):
            ind16T[jt * 128 + kk, 2 * jt + (kk >= 64)] = 1.0

    ind16 = np.zeros((16, DIM), f16)
    for jt in range(NJ):
        for m in range(128):
            ind16[2 * jt + (m >= 64), jt * 128 + m] = 1.0

    Pmat = np.zeros((128, 128), f16)
    for i in range(64):
        Pmat[2 * i + 1, 2 * i] = -1.0  # out[2i] = -q[2i+1]
        Pmat[2 * i, 2 * i + 1] = 1.0  # out[2i+1] = q[2i]

    rows_per_core = _row_assignment(mask)

    in_maps = []
    for core in range(N_CORES):
        b = core // (N_CORES // B)
        rows = rows_per_core[core]
        n = len(rows)
        assert n <= R, f"core {core}: {n} unmasked rows exceed budget {R}"

        pos = np.zeros(R, np.float64)
        pos[:n] = rows

        xc = np.zeros((R, DIM), np.float32)
        xc[:n] = x[b, rows, :]
        xT = xc.T.astype(f16)  # [DIM, R]
        xTc = np.concatenate(
            [xT[:, i * ST : (i + 1) * ST] for i in range(NCHUNK)], axis=0
        )
        xTc = np.ascontiguousarray(xTc)

        angC = pos[:, None] * freq_of_j[None, :]  # [R, 64]
        cosCc = np.cos(angC).astype(f16)
        sinCc = np.sin(angC).astype(np.float32)
        # sign fold for the swap formulation: even j -> -sin, odd j -> +sin
        sinCc[:, 0::2] *= -1.0
        sinCc = sinCc.astype(f16)

        angF = freq_of_j[:, None] * pos[None, :]  # [64, R]
        angF2 = np.concatenate([angF, angF], axis=0)  # [128, R]
        cosFc = np.cos(angF2).astype(f16)
        sinFc = np.sin(angF2).astype(f16)

        flags = np.zeros(R, np.float32)
        flags[:n] = 1.0
        maskCc = np.ascontiguousarray(flags.reshape(NC_T, 128).T)  # [128, NC_T]

        in_maps.append(
            {
                "xTc": xTc,
                "WkT": WkT,
                "WvT": WvT,
                "WqT": WqT,
                "WoT": WoT,
                "cosC": cosCc,
                "sinC": sinCc,
                "cosF": cosFc,
                "sinF": sinFc,
                "maskC": maskCc,
                "ind16T": ind16T,
                "ind16": ind16,
                "Pmat": Pmat,
            }
        )
    return in_maps


def assemble_output(results, mask):
    rows_per_core = _row_assignment(mask)
    out = np.zeros((B, C, DIM), np.float32)
    for core in range(N_CORES):
        b = core // (N_CORES // B)
        rows = rows_per_core[core]
        n = len(rows)
        o = results[core]["out"]  # [DIM, R] f16
        out[b, rows, :] = o[:, :n].T.astype(np.float32)
    return out


def kernel(x, mask, Wq, Wk, Wv, Wo, norm_const):
    nc = _get_nc()
    in_maps = make_in_maps(x, mask, Wq, Wk, Wv, Wo, norm_const)
    res = run_bass_kernel_spmd(nc, in_maps, list(range(N_CORES)))
    return assemble_output(res.results, mask)


# revision 17
# speedup vs baseline: 1.0759x; 1.0250x over previous
"""Trainium2 Bass kernel for nn_Attention_43413529428606 (linear attention
with l2-normed q/k, interleaved RoPE, mask, per-head power scaling).

v2: mask-compacted rows.  Masked rows contribute nothing (k,q masked; kv
only sums unmasked rows) so the host gathers each batch's unmasked rows
(~4.1k of 8192) and strides them across the 4 cores of that batch's
group; each core processes a fixed budget of 1152 rows (real rows
zero-padded, pad flag in maskC).  This cuts every row-proportional GEMM
by ~44% vs the 2048-row dense split.

Other changes vs v1: activation engine restricted to {Copy, Sqrt} (one
act-table set, no reload thrash; rsqrt = DVE reciprocal + Act sqrt),
element-wise work spread across DVE/Pool/Act, x DMA'd in c-chunks so the
first k-projection starts early, norms matmuls emitted one j-tile late,
output stored f16.

Self-contained: hardcodes all shapes; no sibling imports.
"""

import sys

for _p in ("/opt/trn_rl_repo",):
    if _p not in sys.path:
        sys.path.append(_p)

from contextlib import ExitStack

import numpy as np

import concourse.bass as bass
import concourse.bacc as bacc
import concourse.tile as tile
from concourse import mybir
from concourse.bass_utils import run_bass_kernel_spmd

F32 = mybir.dt.float32
F16 = mybir.dt.float16

DIM = 1024
H = 16
HD = 64
B = 2
C = 8192
ROPE_THETA = 10000.0

N_CORES = 8
R = 1152  # padded unmasked-row budget per core (~1037 real at seed 0)
NC_T = R // 128  # 9 c-tiles of 128 (phase A)
ST = 384  # phase-B supertile width
NST = R // ST  # 3
NCHUNK = 3  # x DMA chunks of ST columns
ND = DIM // 128  # 8 d-chunks
NJ = DIM // 128  # 8 j-tiles
NPAIR = H // 2  # 8 head pairs

Copy = mybir.ActivationFunctionType.Copy
Sqrt = mybir.ActivationFunctionType.Sqrt
MUL = mybir.AluOpType.mult
ADD = mybir.AluOpType.add


def build_nc(sim_mode=False, phases="ABC", reps=1, no_collective=False):
    nc = bacc.Bacc(
        "TRN2",
        target_bir_lowering=False,
        debug=False,
        num_devices=1 if sim_mode else N_CORES,
    )

    # ---- DRAM parameters (per-core shapes, fp16 data path) ----
    # x stored c-chunk-major: chunk i is xT[:, i*ST:(i+1)*ST], stacked on
    # axis 0 -> [NCHUNK*DIM, ST]; each chunk DMA is fully contiguous.
    xTc = nc.dram_tensor("xTc", [NCHUNK * DIM, ST], F16, kind="ExternalInput").ap()
    WkT = nc.dram_tensor("WkT", [DIM, DIM], F16, kind="ExternalInput").ap()
    WvT = nc.dram_tensor("WvT", [DIM, DIM], F16, kind="ExternalInput").ap()
    WqT = nc.dram_tensor("WqT", [DIM, DIM], F16, kind="ExternalInput").ap()
    WoT = nc.dram_tensor("WoT", [DIM, DIM], F16, kind="ExternalInput").ap()
    cosC = nc.dram_tensor("cosC", [R, HD], F16, kind="ExternalInput").ap()
    sinC = nc.dram_tensor("sinC", [R, HD], F16, kind="ExternalInput").ap()
    cosF = nc.dram_tensor("cosF", [128, R], F16, kind="ExternalInput").ap()
    sinF = nc.dram_tensor("sinF", [128, R], F16, kind="ExternalInput").ap()
    maskC = nc.dram_tensor("maskC", [128, NC_T], F32, kind="ExternalInput").ap()
    ind16T = nc.dram_tensor("ind16T", [DIM, 16], F16, kind="ExternalInput").ap()
    ind16 = nc.dram_tensor("ind16", [16, DIM], F16, kind="ExternalInput").ap()
    Pmat = nc.dram_tensor("Pmat", [128, 128], F16, kind="ExternalInput").ap()

    kv_in_d = nc.dram_tensor("kv_in_d", [128, NPAIR * 128], F16)
    kv_out_d = nc.dram_tensor("kv_out_d", [128, NPAIR * 128], F16)

    out_d = nc.dram_tensor("out", [DIM, R], F16, kind="ExternalOutput").ap()

    def blkview(dram_ap, csl):
        return dram_ap.rearrange("(t p) c -> p t c", p=128)[:, :, csl]

    with tile.TileContext(nc) as tc:
        with ExitStack() as ctx:
            consts = ctx.enter_context(tc.tile_pool(name="consts", bufs=1))
            kvblk_pool = ctx.enter_context(tc.tile_pool(name="kvblk", bufs=1))

            cosC_t = consts.tile([128, NC_T * HD], F16, tag="cosC")
            sinC_t = consts.tile([128, NC_T * HD], F16, tag="sinC")
            nc.gpsimd.dma_start(
                out=cosC_t[:].rearrange("p (t f) -> p t f", t=NC_T),
                in_=cosC[:].rearrange("(t p) f -> p t f", p=128),
            )
            nc.gpsimd.dma_start(
                out=sinC_t[:].rearrange("p (t f) -> p t f", t=NC_T),
                in_=sinC[:].rearrange("(t p) f -> p t f", p=128),
            )
            maskC_t = consts.tile([128, NC_T], F32, tag="maskC")
            ind16T_t = consts.tile([128, NJ * 16], F16, tag="ind16T")
            ind16_t = consts.tile([16, DIM], F16, tag="ind16")
            P_t = consts.tile([128, 128], F16, tag="Pmat")
            cosF_t = consts.tile([128, R], F16, tag="cosF")
            sinF_t = consts.tile([128, R], F16, tag="sinF")
            nc.gpsimd.dma_start(out=maskC_t[:], in_=maskC[:])
            nc.gpsimd.dma_start(
                out=ind16T_t[:].rearrange("p (t f) -> p t f", t=NJ),
                in_=ind16T[:].rearrange("(t p) f -> p t f", p=128),
            )
            nc.gpsimd.dma_start(out=ind16_t[:], in_=ind16[:])
            nc.gpsimd.dma_start(out=P_t[:], in_=Pmat[:])
            nc.gpsimd.dma_start(out=cosF_t[:], in_=cosF[:])
            nc.gpsimd.dma_start(out=sinF_t[:], in_=sinF[:])

            for _rep in range(reps):
              with ExitStack() as ctxX:
                xpool = ctxX.enter_context(tc.tile_pool(name="xpool", bufs=1))
                wpool = ctxX.enter_context(tc.tile_pool(name="wpool", bufs=1))

                xT_all = xpool.tile([128, ND * R], F16, tag="xT")
                wk_all = wpool.tile([128, ND * DIM], F16, tag="wk")
                wv_all = wpool.tile([128, ND * DIM], F16, tag="wv")
                wq_all = wpool.tile([128, ND * DIM], F16, tag="wq")
                wo_all = wpool.tile([128, ND * DIM], F16, tag="wo")

                xview = xT_all[:].rearrange("p (t c) -> p t c", t=ND)

                # x chunk 0 + wk first (gates the first k matmuls), then
                # the rest; sync and scalar queues in parallel.
                def load_xchunk(ci):
                    nc.sync.dma_start(
                        out=xview[:, :, ci * ST : (ci + 1) * ST],
                        in_=xTc[ci * DIM : (ci + 1) * DIM, :].rearrange(
                            "(t p) c -> p t c", p=128
                        ),
                    )

                def load_w(wt, wsrc, eng):
                    eng.dma_start(
                        out=wt[:].rearrange("p (t f) -> p t f", t=ND),
                        in_=wsrc[:].rearrange("(t p) f -> p t f", p=128),
                    )

                load_xchunk(0)
                load_w(wk_all, WkT, nc.scalar)
                load_w(wv_all, WvT, nc.sync)
                load_xchunk(1)
                load_xchunk(2)
                load_w(wq_all, WqT, nc.scalar)
                load_w(wo_all, WoT, nc.scalar)

                def xsl(dc, csl):
                    lo = dc * R
                    return xT_all[:, lo + csl.start : lo + csl.stop]

                # ========= Phase A: k/v proj + process + kv Grams ==========
                with ExitStack() as ctxA:
                  if "A" in phases:
                    psA = ctxA.enter_context(
                        tc.tile_pool(name="psA", bufs=3, space="PSUM")
                    )
                    pskv = ctxA.enter_context(
                        tc.tile_pool(name="pskv", bufs=1, space="PSUM")
                    )
                    sbA = ctxA.enter_context(tc.tile_pool(name="sbA", bufs=3))
                    sb1 = ctxA.enter_context(tc.tile_pool(name="sb1", bufs=2))
                    smA = ctxA.enter_context(tc.tile_pool(name="smA", bufs=2))

                    kv_ps = pskv.tile([128, NPAIR * 128], F32, tag="kvps")
                    kv_pending = []

                    # On HW start=True zeroes the whole PSUM bank, so only
                    # the first pair written to each bank may carry it.
                    def _emit_kv(item):
                        ct_, khat_, v_ = item
                        for p in range(NPAIR):
                            ps_ = slice(p * 128, (p + 1) * 128)
                            nc.tensor.matmul(
                                kv_ps[:, ps_],
                                v_[:, ps_],
                                khat_[:, ps_],
                                start=(
                                    True
                                    if sim_mode
                                    else (ct_ == 0 and p % 4 == 0)
                                ),
                                stop=(
                                    True if sim_mode else (ct_ == NC_T - 1)
                                ),
                            )

                    for ct in range(NC_T):
                        cs = slice(ct * 128, (ct + 1) * 128)
                        k_ps = psA.tile([128, DIM], F32, tag="proj_ps")
                        v_ps = psA.tile([128, DIM], F32, tag="proj_ps")
                        for half in range(2):
                            js = slice(half * 512, (half + 1) * 512)
                            for dc in range(ND):
                                nc.tensor.matmul(
                                    k_ps[:, js],
                                    xsl(dc, cs),
                                    wk_all[
                                        :, dc * DIM + js.start : dc * DIM + js.stop
                                    ],
                                    start=(dc == 0),
                                    stop=(dc == ND - 1),
                                )
                        for half in range(2):
                            js = slice(half * 512, (half + 1) * 512)
                            for dc in range(ND):
                                nc.tensor.matmul(
                                    v_ps[:, js],
                                    xsl(dc, cs),
                                    wv_all[
                                        :, dc * DIM + js.start : dc * DIM + js.stop
                                    ],
                                    start=(dc == 0),
                                    stop=(dc == ND - 1),
                                )

                        # k evict first: it gates the norm/rope chain
                        k_sb = sbA.tile([128, DIM], F16, tag="k_sb")
                        nc.scalar.activation(k_sb[:], k_ps[:], Copy)
                        # v evict with mask fold (per-partition scale)
                        v_sb = sbA.tile([128, DIM], F16, tag="v_sb")
                        nc.scalar.activation(
                            v_sb[:], v_ps[:], Copy, scale=maskC_t[:, ct : ct + 1]
                        )

                        sq = sbA.tile([128, DIM], F16, tag="sq")
                        nc.vector.tensor_mul(sq[:], k_sb[:], k_sb[:])
                        red = smA.tile([128, H], F32, tag="red")
                        nc.vector.tensor_reduce(
                            red[:],
                            sq[:].rearrange("p (h f) -> p h f", h=H),
                            mybir.AxisListType.X,
                            ADD,
                        )
                        # rsqrt = sqrt(1/(red+eps)); eps guards zero pads
                        rede = smA.tile([128, H], F32, tag="rede")
                        nc.vector.tensor_scalar_add(rede[:], red[:], 1e-4)
                        inv = smA.tile([128, H], F32, tag="inv")
                        nc.vector.reciprocal(inv[:], rede[:])
                        rs = smA.tile([128, H], F32, tag="rs")
                        nc.scalar.activation(rs[:], inv[:], Sqrt)
                        rsm = smA.tile([128, H], F32, tag="rsm")
                        nc.vector.tensor_scalar_mul(
                            rsm[:], rs[:], maskC_t[:, ct : ct + 1]
                        )

                        cosb = (
                            cosC_t[:, ct * HD : (ct + 1) * HD]
                            .unsqueeze(1)
                            .broadcast_to([128, H, HD])
                        )
                        sinb4 = (
                            sinC_t[:, ct * HD : (ct + 1) * HD]
                            .rearrange("p (g two) -> p g two", two=2)
                            .unsqueeze(1)
                            .broadcast_to([128, H, HD // 2, 2])
                        )
                        k3 = k_sb[:].rearrange("p (h f) -> p h f", h=H)
                        k_sw = k_sb[:].rearrange(
                            "p (h g two) -> p h g two", h=H, two=2
                        )[:, :, :, ::-1]

                        m1 = sb1.tile([128, DIM], F16, tag="m1")
                        nc.vector.tensor_tensor(
                            m1[:].rearrange("p (h f) -> p h f", h=H), k3, cosb, MUL
                        )
                        m2 = sb1.tile([128, DIM], F16, tag="m2")
                        nc.gpsimd.tensor_tensor(
                            m2[:].rearrange("p (h g two) -> p h g two", h=H, two=2),
                            k_sw,
                            sinb4,
                            MUL,
                        )
                        s = sb1.tile([128, DIM], F16, tag="s")
                        nc.vector.tensor_tensor(s[:], m1[:], m2[:], ADD)
                        khat = sbA.tile([128, DIM], F16, tag="khat")
                        rsb = rsm[:].unsqueeze(2).broadcast_to([128, H, HD])
                        nc.vector.tensor_tensor(
                            khat[:].rearrange("p (h f) -> p h f", h=H),
                            s[:].rearrange("p (h f) -> p h f", h=H),
                            rsb,
                            MUL,
                        )

                        # kv Grams are issued one iteration late (software
                        # pipelining) so PE never waits on the khat chain
                        kv_pending.append((ct, khat, v_sb))
                        if len(kv_pending) > 2:
                            _emit_kv(kv_pending.pop(0))

                    while kv_pending:
                        _emit_kv(kv_pending.pop(0))

                    # evict kv partials and run the collective
                    kv_sb = sbA.tile([128, NPAIR * 128], F16, tag="kv_sb")
                    nc.vector.tensor_copy(kv_sb[:], kv_ps[:])
                    nc.sync.dma_start(out=kv_in_d.ap(), in_=kv_sb[:])
                    if sim_mode or no_collective:
                        # stand-in for the AllReduce so TimelineSim can run
                        # (no_collective: HW timing diagnostic, wrong output)
                        nc.sync.dma_start(out=kv_out_d.ap(), in_=kv_in_d.ap())
                    else:
                        nc.gpsimd.collective_compute(
                            "AllReduce",
                            ADD,
                            replica_groups=[[0, 1, 2, 3], [4, 5, 6, 7]],
                            ins=[kv_in_d.ap().opt()],
                            outs=[kv_out_d.ap().opt()],
                        )

                # kvT: load reduced Grams (already f16), zero the cross-head
                # 64-blocks, then fold Wo once: kvWo[j, e] = sum_j' kvT[j', j]
                # * WoT[j', e].  Out-proj then contracts qh directly with
                # kvWo.  The PE-side fold is emitted LAZILY at the first
                # _emit_attn_out so two supertiles of q-side work sit ahead
                # of it in the in-order PE queue, covering the AllReduce.
                kvWo = kvblk_pool.tile([128, NJ * DIM], F16, tag="kvWo")
                if "C" in phases:
                    kvT = kvblk_pool.tile([128, NPAIR * 128], F16, tag="kvT")
                    kvf = kvblk_pool.tile([128, NPAIR * 128], F16, tag="kvf")
                    nc.scalar.dma_start(out=kvf[:], in_=kv_out_d.ap())
                    nc.gpsimd.memset(kvT[:], 0.0)
                    # top-left diag blocks of each pair, then bottom-right
                    nc.gpsimd.tensor_copy(
                        kvT[0:64, :].rearrange("p (t f) -> p t f", t=NPAIR)[
                            :, :, 0:64
                        ],
                        kvf[0:64, :].rearrange("p (t f) -> p t f", t=NPAIR)[
                            :, :, 0:64
                        ],
                    )
                    nc.gpsimd.tensor_copy(
                        kvT[64:128, :].rearrange("p (t f) -> p t f", t=NPAIR)[
                            :, :, 64:128
                        ],
                        kvf[64:128, :].rearrange("p (t f) -> p t f", t=NPAIR)[
                            :, :, 64:128
                        ],
                    )

                # ==== Fused phase B+C: q proj/norm/rope + attn + out proj ===
                with ExitStack() as ctxB:
                  if "B" in phases and "C" in phases:
                    psB = ctxB.enter_context(
                        tc.tile_pool(name="psB", bufs=2, space="PSUM")
                    )
                    psRR = ctxB.enter_context(
                        tc.tile_pool(name="psRR", bufs=2, space="PSUM")
                    )
                    psN = ctxB.enter_context(
                        tc.tile_pool(name="psN", bufs=1, space="PSUM")
                    )
                    psW = ctxB.enter_context(
                        tc.tile_pool(name="psW", bufs=1, space="PSUM")
                    )
                    psO = ctxB.enter_context(
                        tc.tile_pool(name="psO", bufs=2, space="PSUM")
                    )
                    sbB = ctxB.enter_context(tc.tile_pool(name="sbB", bufs=3))
                    sbS = ctxB.enter_context(
                        tc.tile_pool(name="sbS", bufs=2 * NJ)
                    )
                    sbQ = ctxB.enter_context(tc.tile_pool(name="sbQ", bufs=2))
                    sbQH = ctxB.enter_context(
                        tc.tile_pool(name="sbQH", bufs=NST)
                    )

                    kvwo_built = [False]

                    def _build_kvwo():
                        if kvwo_built[0]:
                            return
                        kvwo_built[0] = True
                        for jt in range(NJ):
                            for half in range(2):
                                js = slice(half * 512, (half + 1) * 512)
                                w_ps = psW.tile([128, 512], F32, tag="w_ps")
                                nc.tensor.matmul(
                                    w_ps[:],
                                    kvT[:, jt * 128 : (jt + 1) * 128],
                                    wo_all[
                                        :, jt * DIM + js.start : jt * DIM + js.stop
                                    ],
                                    start=True,
                                    stop=True,
                                )
                                dst = kvWo[
                                    :,
                                    jt * DIM + js.start : jt * DIM + js.stop,
                                ]
                                if half % 2 == 0:
                                    nc.scalar.activation(dst, w_ps[:], Copy)
                                else:
                                    nc.vector.tensor_copy(dst, w_ps[:])

                    def _emit_attn_out(item):
                        _build_kvwo()
                        ct_, qh_ = item
                        cs_ = slice(ct_ * ST, (ct_ + 1) * ST)
                        o_all = sbQ.tile([128, NJ * ST], F16, tag="o_all")
                        for et in range(NJ):
                            elo = et * 128
                            o_ps = psO.tile([128, ST], F32, tag="o_ps")
                            for jt in range(NJ):
                                nc.tensor.matmul(
                                    o_ps[:],
                                    kvWo[
                                        :, jt * DIM + elo : jt * DIM + elo + 128
                                    ],
                                    qh_[:, jt * ST : (jt + 1) * ST],
                                    start=(jt == 0),
                                    stop=(jt == NJ - 1),
                                )
                            if et % 2 == 0:
                                nc.scalar.activation(
                                    o_all[:, et * ST : (et + 1) * ST], o_ps[:], Copy
                                )
                            else:
                                nc.vector.tensor_copy(
                                    o_all[:, et * ST : (et + 1) * ST], o_ps[:]
                                )
                        nc.sync.dma_start(
                            out=blkview(out_d, cs_),
                            in_=o_all[:].rearrange("p (t c) -> p t c", t=NJ),
                        )

                    at_pending = []
                    for ct in range(NST):
                        cs = slice(ct * ST, (ct + 1) * ST)
                        norms_ps = psN.tile([16, ST], F32, tag="norms")
                        qh_all = sbQH.tile([128, NJ * ST], F16, tag="qhall")
                        q_sbs = []
                        sq_pending = []

                        def _emit_norms(item):
                            jt_, sq_ = item
                            nc.tensor.matmul(
                                norms_ps[:],
                                ind16T_t[:, jt_ * 16 : (jt_ + 1) * 16],
                                sq_[:],
                                start=(jt_ == 0),
                                stop=(jt_ == NJ - 1),
                            )

                        # pass 1: projections + squares + norm accumulation
                        # (norms matmuls one jt late: PE never waits on sq)
                        for jt in range(NJ):
                            jlo = jt * 128
                            q_ps = psB.tile([128, ST], F32, tag="q_ps")
                            for dc in range(ND):
                                nc.tensor.matmul(
                                    q_ps[:],
                                    wq_all[
                                        :, dc * DIM + jlo : dc * DIM + jlo + 128
                                    ],
                                    xsl(dc, cs),
                                    start=(dc == 0),
                                    stop=(dc == ND - 1),
                                )
                            q_sb = sbS.tile([128, ST], F16, tag="q_sb")
                            nc.scalar.activation(q_sb[:], q_ps[:], Copy)
                            sq = sbB.tile([128, ST], F16, tag="sqB")
                            nc.vector.tensor_mul(sq[:], q_sb[:], q_sb[:])
                            sq_pending.append((jt, sq))
                            if len(sq_pending) > 1:
                                _emit_norms(sq_pending.pop(0))
                            q_sbs.append(q_sb)
                        while sq_pending:
                            _emit_norms(sq_pending.pop(0))

                        # rsqrt = sqrt(1/(norms+eps))
                        ne = sbB.tile([16, ST], F32, tag="ne")
                        nc.vector.tensor_scalar_add(ne[:], norms_ps[:], 1e-4)
                        inv16 = sbB.tile([16, ST], F32, tag="inv16")
                        nc.vector.reciprocal(inv16[:], ne[:])
                        rs16 = sbB.tile([16, ST], F16, tag="rs16")
                        nc.scalar.activation(rs16[:], inv16[:], Sqrt)

                        # pass 2: rot matmuls first (independent of rs16),
                        # then rep broadcast matmuls + the rope/scale chain
                        for jt in range(NJ):
                            rot_ps = psRR.tile([128, ST], F32, tag="rotrep")
                            nc.tensor.matmul(
                                rot_ps[:], P_t[:], q_sbs[jt][:],
                                start=True, stop=True,
                            )
                            rep_ps = psRR.tile([128, ST], F32, tag="rotrep")
                            nc.tensor.matmul(
                                rep_ps[:],
                                ind16_t[:, jt * 128 : (jt + 1) * 128],
                                rs16[:],
                                start=True,
                                stop=True,
                            )

                            t1 = sbB.tile([128, ST], F16, tag="t1")
                            nc.vector.tensor_tensor(
                                t1[:], q_sbs[jt][:], cosF_t[:, cs], MUL
                            )
                            t2 = sbB.tile([128, ST], F16, tag="t2")
                            nc.vector.tensor_tensor(
                                t2[:], rot_ps[:], sinF_t[:, cs], MUL
                            )
                            sB = sbB.tile([128, ST], F16, tag="sB")
                            nc.vector.tensor_tensor(sB[:], t1[:], t2[:], ADD)
                            nc.vector.tensor_tensor(
                                qh_all[:, jt * ST : (jt + 1) * ST],
                                sB[:],
                                rep_ps[:],
                                MUL,
                            )

                        at_pending.append((ct, qh_all))
                        if len(at_pending) > 1:
                            _emit_attn_out(at_pending.pop(0))

                    while at_pending:
                        _emit_attn_out(at_pending.pop(0))

    nc.compile()
    return nc


_NC_CACHE = None


def _get_nc():
    global _NC_CACHE
    if _NC_CACHE is None:
        _NC_CACHE = build_nc()
    return _NC_CACHE


def _row_assignment(mask):
    """Per-core unmasked row indices: batch group b gets cores 4b..4b+3,
    rows strided so counts differ by <=1."""
    rows_per_core = []
    for b in range(B):
        idx = np.where(np.asarray(mask[b]) != 0)[0]
        for cc in range(N_CORES // B):
            rows_per_core.append(idx[cc :: N_CORES // B])
    return rows_per_core


def make_in_maps(x, mask, Wq, Wk, Wv, Wo, norm_const):
    x = np.asarray(x, np.float32)
    mask = np.asarray(mask)
    Wq = np.asarray(Wq, np.float32)
    Wk = np.asarray(Wk, np.float32)
    Wv = np.asarray(Wv, np.float32)
    Wo = np.asarray(Wo, np.float32)
    norm_const = np.asarray(norm_const, np.float32).reshape(H)

    sig = 1.0 / (1.0 + np.exp(-norm_const.astype(np.float64)))
    svec = np.float64(C) ** (-sig)  # [H]
    s_cols = np.repeat(svec, HD)  # [DIM]

    f16 = np.float16
    WkT = np.ascontiguousarray(Wk.T).astype(f16)
    WvT = np.ascontiguousarray((Wv * s_cols[:, None].astype(np.float32)).T).astype(
        f16
    )
    WqT = np.ascontiguousarray(Wq.T).astype(f16)
    WoT = np.ascontiguousarray(Wo.T).astype(f16)

    inv_freq = 1.0 / (
        ROPE_THETA ** (np.arange(0, HD, 2, dtype=np.float64) / HD)
    )  # [32]
    freq_of_j = np.repeat(inv_freq, 2)  # [64] interleaved

    ind16T = np.zeros((DIM, 16), f16)
    for jt in range(NJ):
        for kk in range(128):
            ind16T[jt * 128 + kk, 2 * jt + (kk >= 64)] = 1.0

    ind16 = np.zeros((16, DIM), f16)
    for jt in range(NJ):
        for m in range(128):
            ind16[2 * jt + (m >= 64), jt * 128 + m] = 1.0

    Pmat = np.zeros((128, 128), f16)
    for i in range(64):
        Pmat[2 * i + 1, 2 * i] = -1.0  # out[2i] = -q[2i+1]
        Pmat[2 * i, 2 * i + 1] = 1.0  # out[2i+1] = q[2i]

    rows_per_core = _row_assignment(mask)

    in_maps = []
    for core in range(N_CORES):
        b = core // (N_CORES // B)
        rows = rows_per_core[core]
        n = len(rows)
        assert n <= R, f"core {core}: {n} unmasked rows exceed budget {R}"

        pos = np.zeros(R, np.float64)
        pos[:n] = rows

        xc = np.zeros((R, DIM), np.float32)
        xc[:n] = x[b, rows, :]
        xT = xc.T.astype(f16)  # [DIM, R]
        xTc = np.concatenate(
            [xT[:, i * ST : (i + 1) * ST] for i in range(NCHUNK)], axis=0
        )
        xTc = np.ascontiguousarray(xTc)

        angC = pos[:, None] * freq_of_j[None, :]  # [R, 64]
        cosCc = np.cos(angC).astype(f16)
        sinCc = np.sin(angC).astype(np.float32)
        # sign fold for the swap formulation: even j -> -sin, odd j -> +sin
        sinCc[:, 0::2] *= -1.0
        sinCc = sinCc.astype(f16)

        angF = freq_of_j[:, None] * pos[None, :]  # [64, R]
        angF2 = np.concatenate([angF, angF], axis=0)  # [128, R]
        cosFc = np.cos(angF2).astype(f16)
        sinFc = np.sin(angF2).astype(f16)

        flags = np.zeros(R, np.float32)
        flags[:n] = 1.0
        maskCc = np.ascontiguousarray(flags.reshape(NC_T, 128).T)  # [128, NC_T]

        in_maps.append(
            {
                "xTc": xTc,
                "WkT": WkT,
                "WvT": WvT,
                "WqT": WqT,
                "WoT": WoT,
                "cosC": cosCc,
                "sinC": sinCc,
                "cosF": cosFc,
                "sinF": sinFc,
                "maskC": maskCc,
                "ind16T": ind16T,
                "ind16": ind16,
                "Pmat": Pmat,
            }
        )
    return in_maps


def assemble_output(results, mask):
    rows_per_core = _row_assignment(mask)
    out = np.zeros((B, C, DIM), np.float32)
    for core in range(N_CORES):
        b = core // (N_CORES // B)
        rows = rows_per_core[core]
        n = len(rows)
        o = results[core]["out"]  # [DIM, R] f16
        out[b, rows, :] = o[:, :n].T.astype(np.float32)
    return out


def kernel(x, mask, Wq, Wk, Wv, Wo, norm_const):
    nc = _get_nc()
    in_maps = make_in_maps(x, mask, Wq, Wk, Wv, Wo, norm_const)
    res = run_bass_kernel_spmd(nc, in_maps, list(range(N_CORES)))
    return assemble_output(res.results, mask)
